# revision 16
# baseline (speedup 1.0000x reference)
"""Trainium2 Bass kernel for ChromophoreSolventGNN (2x GCNConv + BN + mean-pool + MLP head).

Strategy (8 NeuronCores, SPMD), v3 "narrow blocks":
  - Destination-shard nodes: core c owns [c*2560, (c+1)*2560), split into 80
    groups of 32 destinations (16 groups per 512-wide PSUM window, 5 windows).
  - Edges are sorted by destination group. A chunk = 128 edge slots feeding
    ONE 32-dst group: matmul(out=psum[:, g*32:+32], lhsT=data[128slot, F],
    rhs=oh[128slot, 32], start/stop per group). PE cost ~1 cycle/slot
    (weight-load bound) instead of 512 cols/chunk; oh bytes collapse ~16x.
  - L1 slot data = host-pregathered x rows with BOTH norms folded (fp16,
    exact); oh1 is a pure 0/1 fp8 mask.
  - L2 slot data = dma_gather'd h1 rows (fp16); edge norms ride in the fp8
    oh. Gathers use prepare_only descriptors + per-segment trigger_dma so
    desc-gen and transfers pipeline off the GpSimd engine.
  - L2 edges split local-source/remote-source per group: local chunks gather
    from own h1loc (no AllGather needed) and run DURING the h1 AllGather;
    remote chunks stream after it.
  - BN stats via per-feature free-dim reductions; global stats via small
    AllGather + local reduce. Mean-pool via one-hot matmul (1/count folded,
    host-built f16); pooled sums AllReduce'd; MLP head replicated.
"""

import numpy as np
import ml_dtypes

import concourse.bass as bass
import concourse.mybir as mybir
from concourse import bacc
from concourse.bass_utils import run_bass_kernel_spmd
from concourse.tile import TileContext

F32 = mybir.dt.float32
F16 = mybir.dt.float16
F8 = mybir.dt.float8e4
I16 = mybir.dt.int16
F8NP = ml_dtypes.float8_e4m3

W = 8            # cores
N = 20000        # nodes
E = 320000       # edges
G = 512          # graphs
F_IN = 64
H1 = 128
H2 = 256
SOLV = 128
EPS = 1e-5

PC = 2560                # nodes per core
NP = W * PC              # padded node count (20480)
GRP = 32                 # destination group width
NGRP = PC // GRP         # groups per core (80)
WIN = 512                # psum window (one bank)
NW = PC // WIN           # windows per core (5)
GPW = WIN // GRP         # groups per window (16)
NB = PC // 128           # 128-node blocks per core (20)
SEG1 = 16                # L1 stream chunks per segment
SEGR = 8                 # L2 remote chunks per gather segment
NGB = 3                  # rotating remote gather buffers
USE_PREP = False         # prepared+triggered remote gathers vs blocking


def _chunk_meta(C):
    """C: per-group chunk counts. Returns [(g, k, nchunks_g)] in slot order."""
    meta = []
    for g, c in enumerate(C):
        for k in range(c):
            meta.append((g, k, c))
    return meta


def _bn_apply_params(nc, tl, st, colw, n_count, g_sb, be_sb, name):
    """From (sum, sumsq) slices compute per-partition scale/shift tiles."""
    mu = tl.tile([128, 1], F32, tag=f"mu{name}")
    nc.vector.tensor_scalar_mul(mu[:], st[:, colw : colw + 1], 1.0 / n_count)
    var = tl.tile([128, 1], F32, tag=f"var{name}")
    nc.vector.tensor_scalar_mul(var[:], st[:, colw + 1 : colw + 2], 1.0 / n_count)
    musq = tl.tile([128, 1], F32, tag=f"musq{name}")
    nc.vector.tensor_tensor(out=musq[:], in0=mu[:], in1=mu[:], op=mybir.AluOpType.mult)
    nc.vector.tensor_tensor(out=var[:], in0=var[:], in1=musq[:], op=mybir.AluOpType.subtract)
    nc.vector.tensor_scalar_add(var[:], var[:], EPS)
    rv = tl.tile([128, 1], F32, tag=f"rv{name}")
    nc.vector.reciprocal(out=rv[:], in_=var[:])
    rstd = tl.tile([128, 1], F32, tag=f"rstd{name}")
    nc.scalar.sqrt(out=rstd[:], in_=rv[:])
    sc = tl.tile([128, 1], F32, tag=f"sc{name}")
    nc.vector.tensor_tensor(out=sc[:], in0=g_sb[:], in1=rstd[:], op=mybir.AluOpType.mult)
    sh = tl.tile([128, 1], F32, tag=f"sh{name}")
    nc.vector.tensor_tensor(out=sh[:], in0=mu[:], in1=sc[:], op=mybir.AluOpType.mult)
    nc.vector.tensor_tensor(out=sh[:], in0=be_sb[:], in1=sh[:], op=mybir.AluOpType.subtract)
    return sc, sh


def _build_program(key):
    C1, C2L, C2R = [list(c) for c in key]
    M1 = _chunk_meta(C1)
    ML = _chunk_meta(C2L)
    MR = _chunk_meta(C2R)
    NCH1, NCHL, NCHR = len(M1), len(ML), len(MR)
    NSEG1 = (NCH1 + SEG1 - 1) // SEG1
    NSEGR = (NCHR + SEGR - 1) // SEGR
    # first/last chunk index per window, L1 and L2-remote. PSUM accumulation
    # start/stop granularity is the 2KB zero region (a whole bank row), so
    # each window bank runs ONE accumulation group: start on the window's
    # first chunk, stop on its last; intermediate columns are zeroed lazily
    # by the pending-zero-on-first-touch mechanism.
    firstw1, lastw1, firstw2, lastw2 = {}, {}, {}, {}
    for i, (g, k, c) in enumerate(M1):
        firstw1.setdefault(g // GPW, i)
        lastw1[g // GPW] = i
    for i, (g, k, c) in enumerate(MR):
        firstw2.setdefault(g // GPW, i)
        lastw2[g // GPW] = i

    nc = bacc.Bacc("TRN2", target_bir_lowering=False, debug=False, num_devices=W,
                   dynamic_dma_scratch_size=32768)

    # ---- external inputs -------------------------------------------------
    xe_d = nc.dram_tensor("xe", [128, NCH1, F_IN], F16, kind="ExternalInput")
    oh1_d = nc.dram_tensor("oh1", [128, NCH1, GRP], F8, kind="ExternalInput")
    ohl_d = nc.dram_tensor("ohl", [128, NCHL, GRP], F8, kind="ExternalInput")
    oh2_d = nc.dram_tensor("oh2", [128, NCHR, GRP], F8, kind="ExternalInput")
    gidxl_d = nc.dram_tensor("gidxl", [128, NCHL * 8], I16, kind="ExternalInput")
    gidxr_d = nc.dram_tensor("gidxr", [128, NCHR * 8], I16, kind="ExternalInput")
    pool_d = nc.dram_tensor("poolh", [128, NB, G], F16, kind="ExternalInput")
    sfT_d = nc.dram_tensor("sfT", [SOLV, G], F16, kind="ExternalInput")
    w1_d = nc.dram_tensor("w1", [F_IN, H1], F16, kind="ExternalInput")
    w2_d = nc.dram_tensor("w2", [H1, H2], F16, kind="ExternalInput")
    ws_d = nc.dram_tensor("ws", [SOLV, 128], F16, kind="ExternalInput")
    wf1_d = nc.dram_tensor("wf1", [128, 3, 128], F16, kind="ExternalInput")
    wf2_d = nc.dram_tensor("wf2", [128, 1], F16, kind="ExternalInput")
    g1_d = nc.dram_tensor("g1", [128, 1], F32, kind="ExternalInput")
    be1_d = nc.dram_tensor("be1", [128, 1], F32, kind="ExternalInput")
    g2_d = nc.dram_tensor("g2", [128, 2], F32, kind="ExternalInput")
    be2_d = nc.dram_tensor("be2", [128, 2], F32, kind="ExternalInput")
    gf1_d = nc.dram_tensor("gf1", [128, 1], F32, kind="ExternalInput")
    bef1_d = nc.dram_tensor("bef1", [128, 1], F32, kind="ExternalInput")
    bs_d = nc.dram_tensor("bs", [128, 1], F32, kind="ExternalInput")
    bf2_d = nc.dram_tensor("bf2", [1, 1], F32, kind="ExternalInput")

    out_d = nc.dram_tensor("out", [G, 1], F32, kind="ExternalOutput")

    # ---- internal DRAM ---------------------------------------------------
    h1loc_d = nc.dram_tensor("h1loc", [PC, H1], F16)
    h1full_d = nc.dram_tensor("h1full", [NP, H1], F16, addr_space="Shared")
    bn1i_d = nc.dram_tensor("bn1i", [128, 2], F32)
    bn1o_d = nc.dram_tensor("bn1o", [W * 128, 2], F32, addr_space="Shared")
    bn2i_d = nc.dram_tensor("bn2i", [128, 4], F32)
    bn2o_d = nc.dram_tensor("bn2o", [W * 128, 4], F32, addr_space="Shared")
    pli_d = nc.dram_tensor("pli", [2 * 128, G], F32)
    plo_d = nc.dram_tensor("plo", [2 * 128, G], F32, addr_space="Shared")

    RG = [list(range(W))]

    with TileContext(nc) as tc:
        with tc.tile_pool(name="const", bufs=1) as cst, \
             tc.tile_pool(name="ohs", bufs=3) as ohs, \
             tc.tile_pool(name="xes", bufs=3) as xes, \
             tc.tile_pool(name="scr", bufs=3) as scr, \
             tc.tile_pool(name="winp", bufs=1, space="PSUM") as psw, \
             tc.tile_pool(name="psproj", bufs=2, space="PSUM") as psp, \
             tc.tile_pool(name="pshead", bufs=1, space="PSUM") as psh:

            def load_const(name, dram, shape, dt):
                t = cst.tile(shape, dt, name=name)
                nc.sync.dma_start(out=t[:], in_=dram[:])
                return t

            gidxl_sb = load_const("gidxl_sb", gidxl_d, [128, NCHL * 8], I16)
            gidxr_sb = load_const("gidxr_sb", gidxr_d, [128, NCHR * 8], I16)
            w1_sb = load_const("w1_sb", w1_d, [F_IN, H1], F16)
            w2_sb = load_const("w2_sb", w2_d, [H1, H2], F16)
            ws_sb = load_const("ws_sb", ws_d, [SOLV, 128], F16)
            wf1_sb = load_const("wf1_sb", wf1_d, [128, 3, 128], F16)
            wf2_sb = load_const("wf2_sb", wf2_d, [128, 1], F16)
            g1_sb = load_const("g1_sb", g1_d, [128, 1], F32)
            be1_sb = load_const("be1_sb", be1_d, [128, 1], F32)
            g2_sb = load_const("g2_sb", g2_d, [128, 2], F32)
            be2_sb = load_const("be2_sb", be2_d, [128, 2], F32)
            gf1_sb = load_const("gf1_sb", gf1_d, [128, 1], F32)
            bef1_sb = load_const("bef1_sb", bef1_d, [128, 1], F32)
            bs_sb = load_const("bs_sb", bs_d, [128, 1], F32)
            bf2_sb = load_const("bf2_sb", bf2_d, [1, 1], F32)
            ohl_sb = load_const("ohl_sb", ohl_d, [128, NCHL, GRP], F8)

            # 5 window psum banks, shared by L1 (sequential) then L2 (all open)
            winps = [psw.tile([128, WIN], F32, name=f"winps{w}") for w in range(NW)]

            # ---------- layer 1: narrow-block aggregate ----------
            agg1T16 = cst.tile([F_IN, NW, WIN], F16, name="agg1T16")
            h1Traw = cst.tile([H1, NW, WIN], F16, name="h1Traw")
            s1p = cst.tile([128, 2 * NW], F32, name="s1p")

            def close_l1_window(w):
                nc.vector.tensor_copy(out=agg1T16[:, w, :], in_=winps[w][0:F_IN, :])
                ph = psp.tile([H1, WIN], F32, tag="ph1")
                nc.tensor.matmul(out=ph[:], lhsT=w1_sb[:], rhs=agg1T16[:, w, :],
                                 start=True, stop=True)
                nc.vector.tensor_copy(out=h1Traw[:, w, :], in_=ph[:])
                nc.vector.tensor_reduce(out=s1p[:, w : w + 1], in_=ph[:],
                                        axis=mybir.AxisListType.X, op=mybir.AluOpType.add)
                sq = scr.tile([H1, WIN], F32, tag="sq")
                nc.scalar.square(out=sq[:], in_=ph[:])
                nc.vector.tensor_reduce(out=s1p[:, NW + w : NW + w + 1], in_=sq[:],
                                        axis=mybir.AxisListType.X, op=mybir.AluOpType.add)

            for s in range(NSEG1):
                nch = min(SEG1, NCH1 - s * SEG1)
                ohseg = ohs.tile([128, SEG1, GRP], F8, tag="oh1s")
                nc.sync.dma_start(out=ohseg[:, 0:nch, :],
                                  in_=oh1_d[:, s * SEG1 : s * SEG1 + nch, :])
                xeseg = xes.tile([128, SEG1, F_IN], F16, tag="xes")
                nc.sync.dma_start(out=xeseg[:, 0:nch, :],
                                  in_=xe_d[:, s * SEG1 : s * SEG1 + nch, :])
                for jj in range(nch):
                    ch = s * SEG1 + jj
                    g, k, cg = M1[ch]
                    w, gc = divmod(g, GPW)
                    nc.tensor.matmul(
                        out=winps[w][0:F_IN, gc * GRP : (gc + 1) * GRP],
                        lhsT=xeseg[:, jj, :],
                        rhs=ohseg[:, jj, :],
                        start=(ch == firstw1[w]),
                        stop=(ch == lastw1[w]),
                    )
                    if ch == lastw1[w]:
                        close_l1_window(w)

            # deferred consts (tail-only) + solvent branch
            pool_sb = load_const("pool_sb", pool_d, [128, NB, G], F16)
            sfT_sb = load_const("sfT_sb", sfT_d, [SOLV, G], F16)
            psv = psh.tile([128, G], F32, tag="hps")
            nc.tensor.matmul(out=psv[:], lhsT=ws_sb[:], rhs=sfT_sb[:],
                             start=True, stop=True)
            solvT = cst.tile([128, G], F16, name="solvT")
            nc.scalar.activation(out=solvT[:], in_=psv[:],
                                 func=mybir.ActivationFunctionType.Relu,
                                 bias=bs_sb[:], scale=1.0)

            # ---------- BN1 global stats ----------
            st1 = cst.tile([128, 2], F32, name="st1")
            nc.vector.tensor_reduce(out=st1[:, 0:1], in_=s1p[:, 0:NW],
                                    axis=mybir.AxisListType.X, op=mybir.AluOpType.add)
            nc.vector.tensor_reduce(out=st1[:, 1:2], in_=s1p[:, NW : 2 * NW],
                                    axis=mybir.AxisListType.X, op=mybir.AluOpType.add)
            nc.gpsimd.dma_start(out=bn1i_d[:], in_=st1[:])
            nc.gpsimd.collective_compute(
                "AllGather", mybir.AluOpType.bypass,
                ins=[bn1i_d[:]], outs=[bn1o_d[:]], replica_groups=RG,
            )
            st1g = cst.tile([128, W, 2], F32, name="st1g")
            nc.sync.dma_start(out=st1g[:],
                              in_=bn1o_d[:].rearrange("(c p) j -> p c j", p=128))
            st1s = cst.tile([128, 2], F32, name="st1s")
            for j in range(2):
                nc.vector.tensor_reduce(out=st1s[:, j : j + 1], in_=st1g[:, :, j],
                                        axis=mybir.AxisListType.X, op=mybir.AluOpType.add)
            sc1, sh1 = _bn_apply_params(nc, cst, st1s, 0, N, g1_sb, be1_sb, "1")

            # apply BN1 + relu -> fp16 feature-major
            h1T16 = cst.tile([H1, NW, WIN], F16, name="h1T16")
            for w in range(NW):
                nc.scalar.activation(out=h1T16[:, w, :], in_=h1Traw[:, w, :],
                                     func=mybir.ActivationFunctionType.Relu,
                                     bias=sh1[:], scale=sc1[:])
            # transpose to node-major, store, AllGather
            h1T16f = h1T16[:].rearrange("p w n -> p (w n)")
            h1nm = cst.tile([128, NB, H1], F16, name="h1nm")
            nc.sync.dma_start_transpose(out=h1nm[:], in_=h1T16f)
            nc.gpsimd.dma_start(
                out=h1loc_d[:].rearrange("(t p) f -> p t f", p=128), in_=h1nm[:]
            )
            nc.gpsimd.collective_compute(
                "AllGather", mybir.AluOpType.bypass,
                ins=[h1loc_d[:]], outs=[h1full_d[:]], replica_groups=RG,
            )

            # ---------- layer 2 ----------
            gsems = [nc.alloc_semaphore(f"gsem{s}") for s in range(NSEGR)]
            # local gather (from own h1loc; overlaps the h1 AllGather).
            # Blocking (non-prepped) so the RAW on the h1loc DMA write is a
            # proper sem wait; it hides inside the AllGather wall time.
            gbl = cst.tile([128, NCHL, H1], F16, name="gbl")
            SEGL = 8
            for s in range(0, NCHL, SEGL):
                nch = min(SEGL, NCHL - s)
                nidx = nch * 128
                nc.gpsimd.dma_gather(
                    out_ap=gbl[:, s : s + nch, :],
                    in_ap=h1loc_d[:],
                    idxs_ap=gidxl_sb[:, s * 8 : s * 8 + nidx // 16],
                    num_idxs=nidx,
                    num_idxs_reg=nidx,
                    elem_size=H1,
                )

            # local-edge matmuls (all windows open; one accumulation group per
            # bank: start only on each window's first local chunk)
            li = 0
            for g in range(NGRP):
                w, gc = divmod(g, GPW)
                for k in range(C2L[g]):
                    nc.tensor.matmul(
                        out=winps[w][:, gc * GRP : (gc + 1) * GRP],
                        lhsT=gbl[:, li, :],
                        rhs=ohl_sb[:, li, :],
                        start=(g == w * GPW and k == 0),
                        stop=False,
                    )
                    li += 1

            agg2T16 = cst.tile([H1, NW, WIN], F16, name="agg2T16")
            h2Traw = cst.tile([128, 2, NW, WIN], F16, name="h2Traw")
            s2p = cst.tile([128, 2, 2 * NW], F32, name="s2p")

            def close_l2_window(w):
                nc.vector.tensor_copy(out=agg2T16[:, w, :], in_=winps[w][:])
                for half in range(2):
                    ph = psp.tile([128, WIN], F32, tag="ph1")
                    nc.tensor.matmul(
                        out=ph[:], lhsT=w2_sb[:, half * 128 : (half + 1) * 128],
                        rhs=agg2T16[:, w, :], start=True, stop=True,
                    )
                    nc.vector.tensor_copy(out=h2Traw[:, half, w, :], in_=ph[:])
                    nc.vector.tensor_reduce(out=s2p[:, half, w : w + 1], in_=ph[:],
                                            axis=mybir.AxisListType.X, op=mybir.AluOpType.add)
                    sq = scr.tile([128, WIN], F32, tag="sq")
                    nc.scalar.square(out=sq[:], in_=ph[:])
                    nc.vector.tensor_reduce(out=s2p[:, half, NW + w : NW + w + 1], in_=sq[:],
                                            axis=mybir.AxisListType.X, op=mybir.AluOpType.add)

            # remote-edge segments: prep+trigger pipelined gathers
            gbr = [cst.tile([128, SEGR, H1], F16, name=f"gbr{i}") for i in range(NGB)]
            for s in range(NSEGR):
                nch = min(SEGR, NCHR - s * SEGR)
                nidx = nch * 128
                gb = gbr[s % NGB]
                if USE_PREP:
                    nc.gpsimd.dma_gather(
                        out_ap=gb[:, 0:nch, :],
                        in_ap=h1full_d[:],
                        idxs_ap=gidxr_sb[:, s * SEGR * 8 : s * SEGR * 8 + nidx // 16],
                        num_idxs=nidx,
                        num_idxs_reg=nidx,
                        elem_size=H1,
                        prepare_only=True,
                        sem=gsems[s],
                        queue_num=0,
                    )
                    nc.gpsimd.trigger_dma(count=None, queue_num=0)
                else:
                    nc.gpsimd.dma_gather(
                        out_ap=gb[:, 0:nch, :],
                        in_ap=h1full_d[:],
                        idxs_ap=gidxr_sb[:, s * SEGR * 8 : s * SEGR * 8 + nidx // 16],
                        num_idxs=nidx,
                        num_idxs_reg=nidx,
                        elem_size=H1,
                    )
                ohseg = ohs.tile([128, SEGR, GRP], F8, tag="oh2s")
                nc.sync.dma_start(out=ohseg[:, 0:nch, :],
                                  in_=oh2_d[:, s * SEGR : s * SEGR + nch, :])
                if USE_PREP:
                    # Tile pre-bumps its DMASW lane sem for prepped gathers
                    # and leaves data sync to the caller's sem: gate the
                    # consuming matmuls on this segment's completion
                    # explicitly (per-segment sems — triggered ring entries
                    # may complete out of order).
                    nc.tensor.wait_ge(gsems[s], 16)
                for jj in range(nch):
                    ch = s * SEGR + jj
                    g, k, cg = MR[ch]
                    w, gc = divmod(g, GPW)
                    nc.tensor.matmul(
                        out=winps[w][:, gc * GRP : (gc + 1) * GRP],
                        lhsT=gb[:, jj, :],
                        rhs=ohseg[:, jj, :],
                        start=False,
                        stop=(ch == lastw2[w]),
                    )
                    if ch == lastw2[w]:
                        close_l2_window(w)

            # ---------- BN2 ----------
            st2 = cst.tile([128, 4], F32, name="st2")
            for half in range(2):
                nc.vector.tensor_reduce(out=st2[:, 2 * half : 2 * half + 1],
                                        in_=s2p[:, half, 0:NW],
                                        axis=mybir.AxisListType.X, op=mybir.AluOpType.add)
                nc.vector.tensor_reduce(out=st2[:, 2 * half + 1 : 2 * half + 2],
                                        in_=s2p[:, half, NW : 2 * NW],
                                        axis=mybir.AxisListType.X, op=mybir.AluOpType.add)
            nc.gpsimd.dma_start(out=bn2i_d[:], in_=st2[:])
            nc.gpsimd.collective_compute(
                "AllGather", mybir.AluOpType.bypass,
                ins=[bn2i_d[:]], outs=[bn2o_d[:]], replica_groups=RG,
            )
            st2g = cst.tile([128, W, 4], F32, name="st2g")
            nc.sync.dma_start(out=st2g[:],
                              in_=bn2o_d[:].rearrange("(c p) j -> p c j", p=128))
            st2s = cst.tile([128, 4], F32, name="st2s")
            for j in range(4):
                nc.vector.tensor_reduce(out=st2s[:, j : j + 1], in_=st2g[:, :, j],
                                        axis=mybir.AxisListType.X, op=mybir.AluOpType.add)

            # apply BN2 + relu, transpose to node-major
            h2app = [agg2T16, h1T16]
            for half in range(2):
                sc2, sh2 = _bn_apply_params(
                    nc, cst, st2s, 2 * half, N,
                    g2_sb[:, half : half + 1], be2_sb[:, half : half + 1], f"2{half}",
                )
                for w in range(NW):
                    nc.scalar.activation(out=h2app[half][:, w, :],
                                         in_=h2Traw[:, half, w, :],
                                         func=mybir.ActivationFunctionType.Relu,
                                         bias=sh2[:], scale=sc2[:])
            h2nm = cst.tile([128, NB, 2, 128], F16, name="h2nm")
            for half in range(2):
                hsrc = h2app[half][:].rearrange("p w n -> p (w n)")
                nc.sync.dma_start_transpose(out=h2nm[:, :, half, :], in_=hsrc)

            # ---------- mean-pool ----------
            poolsb = cst.tile([128, 2, G], F32, name="poolsb")
            for half in range(2):
                pp = psp.tile([128, G], F32, tag="ph1")
                for t in range(NB):
                    nc.tensor.matmul(
                        out=pp[:],
                        lhsT=h2nm[:, t, half, :],
                        rhs=pool_sb[:, t, :],
                        start=(t == 0),
                        stop=(t == NB - 1),
                    )
                nc.vector.tensor_copy(out=poolsb[:, half, :], in_=pp[:])
            nc.gpsimd.dma_start(out=pli_d[:].rearrange("(h p) g -> p h g", p=128),
                                in_=poolsb[:])
            nc.gpsimd.collective_compute(
                "AllReduce", mybir.AluOpType.add,
                ins=[pli_d[:]], outs=[plo_d[:]], replica_groups=RG,
            )
            poolTg = cst.tile([128, 2, G], F32, name="poolTg")
            nc.sync.dma_start(out=poolTg[:],
                              in_=plo_d[:].rearrange("(h p) g -> p h g", p=128))
            poolT16 = cst.tile([128, 2, G], F16, name="poolT16")
            nc.vector.tensor_copy(out=poolT16[:], in_=poolTg[:])

            # ---------- head ----------
            pzf = psh.tile([128, G], F32, tag="hps")
            zins = [poolT16[:, 0, :], poolT16[:, 1, :], solvT[:]]
            for k in range(3):
                nc.tensor.matmul(out=pzf[:], lhsT=wf1_sb[:, k, :], rhs=zins[k],
                                 start=(k == 0), stop=(k == 2))
            st3 = cst.tile([128, 2], F32, name="st3")
            nc.vector.tensor_reduce(out=st3[:, 0:1], in_=pzf[:],
                                    axis=mybir.AxisListType.X, op=mybir.AluOpType.add)
            sq3 = cst.tile([128, G], F32, name="sq3")
            nc.scalar.square(out=sq3[:], in_=pzf[:])
            nc.vector.tensor_reduce(out=st3[:, 1:2], in_=sq3[:],
                                    axis=mybir.AxisListType.X, op=mybir.AluOpType.add)
            sc3, sh3 = _bn_apply_params(nc, cst, st3, 0, G, gf1_sb, bef1_sb, "3")
            zfa = cst.tile([128, G], F16, name="zfa")
            nc.scalar.activation(out=zfa[:], in_=pzf[:],
                                 func=mybir.ActivationFunctionType.Relu,
                                 bias=sh3[:], scale=sc3[:])
            pot = psh.tile([128, G], F32, tag="hps")
            po = pot[0:1, :]
            nc.tensor.matmul(out=po, lhsT=wf2_sb[:], rhs=zfa[:],
                             start=True, stop=True)
            out_sb = cst.tile([1, G], F32, name="out_sb")
            nc.vector.tensor_scalar(
                out=out_sb[:], in0=po, scalar1=bf2_sb[:], scalar2=None,
                op0=mybir.AluOpType.add,
            )
            nc.sync.dma_start(out=out_d[:].rearrange("n o -> o n"), in_=out_sb[:])

    nc.finalize()
    _legalize_waits(nc)
    return nc


def _legalize_waits(nc, max_waits=1):
    """This walrus build rejects instructions with >1-2 sem waits. Hoist the
    excess onto preceding same-engine NoOps (sequencers run in program order)."""
    for fn in nc.m.functions:
        for bb in fn.blocks:
            new_insts = []
            for ins in bb.instructions:
                si = ins.sync_info
                if si is not None and si.on_wait and len(si.on_wait) > max_waits:
                    waits = list(si.on_wait)
                    keep = waits[: max_waits - 1] if max_waits > 1 else []
                    move = waits[len(keep):]
                    keep.append(move.pop())
                    for i, wv in enumerate(move):
                        nop = mybir.InstNoOp(name=f"{ins.name}_ws{i}", ins=[], outs=[],
                                             engine=ins.engine)
                        nop.sync_info = mybir.SyncInfo(on_wait=[wv], on_update=[])
                        new_insts.append(nop)
                        nc.register_instruction(nop, overwrite=True)
                    si.on_wait = keep
                new_insts.append(ins)
            bb.instructions[:] = new_insts


def _slot_fill(order_groups, C, per_core_lists):
    """Given per-core per-group edge index lists and shared chunk counts C,
    return per-core (slot->edge or -1) arrays."""
    S = np.zeros(len(C) + 1, np.int64)
    np.cumsum(np.asarray(C) * 128, out=S[1:])
    nslot = int(S[-1])
    out = []
    for lists in per_core_lists:
        sl = np.full(nslot, -1, np.int64)
        for g, idxs in enumerate(lists):
            sl[S[g] : S[g] + len(idxs)] = idxs
        out.append(sl)
    return out, nslot


def _mk_idx16(vals):
    """[NSLOT] int -> [128, NSLOT/16] int16 wrapped-16 replicated layout."""
    n = len(vals)
    return np.ascontiguousarray(
        np.tile(vals.astype(np.int16).reshape(n // 16, 16).T, (8, 1)))


def _preprocess(x, edge_index, batch, solvent_fingerprint,
                W1, b1, g1, be1, W2, b2, g2, be2,
                Ws, bs, Wf1, bf1, gf1, bef1, Wf2, bf2):
    """Host-side sharding/index preprocessing. Returns (key, in_maps)."""
    edge_index = np.asarray(edge_index)
    batch = np.asarray(batch).astype(np.int64)
    x = np.ascontiguousarray(np.asarray(x, dtype=np.float32))

    loops = np.arange(N, dtype=np.int64)
    row = np.concatenate([edge_index[0].astype(np.int64), loops])
    col = np.concatenate([edge_index[1].astype(np.int64), loops])
    deg = np.bincount(col, minlength=N).astype(np.float32)
    dis = (1.0 / np.sqrt(deg)).astype(np.float32)
    norm = dis[row] * dis[col]

    e_core = col // PC
    g_loc = (col % PC) // GRP          # group within core (0..79)
    dcol = col % GRP                   # column within group
    is_loc = (row // PC) == e_core

    # per-core per-group edge lists (L1: all; L2: local / remote)
    idx_all = [[None] * NGRP for _ in range(W)]
    idx_loc = [[None] * NGRP for _ in range(W)]
    idx_rem = [[None] * NGRP for _ in range(W)]
    cell = e_core * NGRP + g_loc
    order = np.argsort(cell, kind="stable")
    bounds = np.searchsorted(cell[order], np.arange(W * NGRP + 1))
    for c in range(W):
        for g in range(NGRP):
            sel = order[bounds[c * NGRP + g] : bounds[c * NGRP + g + 1]]
            l = sel[is_loc[sel]]
            r = sel[~is_loc[sel]]
            idx_all[c][g] = sel
            idx_loc[c][g] = l
            idx_rem[c][g] = r

    def counts(idx):
        return [max(1, max(int(np.ceil(len(idx[c][g]) / 128)) for c in range(W)))
                for g in range(NGRP)]

    C1 = counts(idx_all)
    C2L = counts(idx_loc)
    C2R = counts(idx_rem)

    sl1, NSLOT1 = _slot_fill(None, C1, idx_all)
    slL, NSLOTL = _slot_fill(None, C2L, idx_loc)
    slR, NSLOTR = _slot_fill(None, C2R, idx_rem)
    NCH1, NCHL, NCHR = NSLOT1 // 128, NSLOTL // 128, NSLOTR // 128

    gcnt = np.bincount(batch, minlength=G).astype(np.float32)
    pw = 1.0 / np.maximum(gcnt, 1.0)

    rep = {
        "sfT": np.ascontiguousarray(
            np.asarray(solvent_fingerprint, np.float32).T).astype(np.float16),
        "w1": np.asarray(W1, np.float32).astype(np.float16),
        "w2": np.asarray(W2, np.float32).astype(np.float16),
        "ws": np.asarray(Ws, np.float32).astype(np.float16),
        "wf1": np.ascontiguousarray(
            np.asarray(Wf1, np.float32).reshape(3, 128, 128).transpose(1, 0, 2)
        ).astype(np.float16),
        "wf2": np.asarray(Wf2, np.float32).reshape(128, 1).astype(np.float16),
        "g1": np.asarray(g1, np.float32).reshape(128, 1),
        "be1": np.asarray(be1, np.float32).reshape(128, 1),
        "g2": np.ascontiguousarray(np.asarray(g2, np.float32).reshape(2, 128).T),
        "be2": np.ascontiguousarray(np.asarray(be2, np.float32).reshape(2, 128).T),
        "gf1": np.asarray(gf1, np.float32).reshape(128, 1),
        "bef1": np.asarray(bef1, np.float32).reshape(128, 1),
        "bs": np.asarray(bs, np.float32).reshape(128, 1),
        "bf2": np.asarray(bf2, np.float32).reshape(1, 1),
    }

    in_maps = []
    for c in range(W):
        s1 = sl1[c]
        v1 = s1 >= 0
        xe = np.zeros((NSLOT1, F_IN), np.float32)
        xe[v1] = x[row[s1[v1]]] * norm[s1[v1]][:, None]
        oh1 = np.zeros((NSLOT1, GRP), np.float32)
        oh1[np.nonzero(v1)[0], dcol[s1[v1]]] = 1.0

        sL = slL[c]
        vL = sL >= 0
        lidx = np.zeros(NSLOTL, np.int64)
        lidx[vL] = row[sL[vL]] % PC
        ohl = np.zeros((NSLOTL, GRP), np.float32)
        ohl[np.nonzero(vL)[0], dcol[sL[vL]]] = norm[sL[vL]]

        sR = slR[c]
        vR = sR >= 0
        ridx = np.zeros(NSLOTR, np.int64)
        ridx[vR] = row[sR[vR]]
        oh2 = np.zeros((NSLOTR, GRP), np.float32)
        oh2[np.nonzero(vR)[0], dcol[sR[vR]]] = norm[sR[vR]]

        # pool one-hot for this core's nodes
        nodes = np.arange(c * PC, (c + 1) * PC)
        pv = np.zeros((PC, G), np.float32)
        real = nodes < N
        pv[real, batch[nodes[real]]] = pw[batch[nodes[real]]]

        mm = dict(rep)
        mm.update({
            "xe": np.ascontiguousarray(
                xe.reshape(NCH1, 128, F_IN).transpose(1, 0, 2)).astype(np.float16),
            "oh1": np.ascontiguousarray(
                oh1.reshape(NCH1, 128, GRP).transpose(1, 0, 2)).astype(F8NP),
            "ohl": np.ascontiguousarray(
                ohl.reshape(NCHL, 128, GRP).transpose(1, 0, 2)).astype(F8NP),
            "oh2": np.ascontiguousarray(
                oh2.reshape(NCHR, 128, GRP).transpose(1, 0, 2)).astype(F8NP),
            "gidxl": _mk_idx16(lidx),
            "gidxr": _mk_idx16(ridx),
            "poolh": np.ascontiguousarray(
                pv.reshape(NB, 128, G).transpose(1, 0, 2)).astype(np.float16),
        })
        in_maps.append(mm)
    return (tuple(C1), tuple(C2L), tuple(C2R)), in_maps


_PROG_CACHE = {}


def _get_program(key):
    if key not in _PROG_CACHE:
        _PROG_CACHE[key] = _build_program(key)
    return _PROG_CACHE[key]


def kernel(**inputs) -> np.ndarray:
    key, in_maps = _preprocess(**inputs)
    nc = _get_program(key)
    res = run_bass_kernel_spmd(nc, in_maps, core_ids=list(range(W)))
    return np.asarray(res.results[0]["out"], dtype=np.float32)
